# revision 1
# baseline (speedup 1.0000x reference)
"""Bass/Tile TRN2 kernel for nn_LzScaleDotAttention (B=8, L=2048, D=512).

Math per batch b:
    S[q,k]   = sum_d Q[q,d] K[k,d]
    E        = exp(S)                       # inputs are pre-scaled small, no max-sub needed
    num[k,d] = sum_q E[q,k] V[q,d]          # = E^T @ V
    den[k]   = sum_q E[q,k]
    mask[k]  = 1.0 if any(V[k,:] != 0) else 0.0
    out[k,d] = num[k,d] * mask[k]*c / (den[k]*mask[k]*c + EPS),  c = 1/sqrt(D)

The renormalisation over the query axis commutes with the E^T@V contraction
(the divisor depends only on k), so the normalised attention matrix is never
materialised: one flash-style pass over q tiles accumulates num (PSUM) and
den (SBUF f32 accumulator + a tiny cross-partition matmul against ones).

Sharding: batch dim (8) across the 8 NeuronCores, one batch per core (SPMD,
no collectives). Matmuls run in float32r (fp32 storage, ~1 cycle/row on the
PE for N=512). Q and K are laid out feature-major ([D, L]) host-side when
sharding, so the device spends no PE cycles transposing operands.
"""

import math
import os
import sys

import numpy as np

for _p in ("/opt/trn_rl_repo", "/root/.axon_site/_ro/trn_rl_repo"):
    if os.path.isdir(_p) and _p not in sys.path:
        sys.path.append(_p)

import concourse.bacc as bacc
import concourse.mybir as mybir
import concourse.tile as tile
from concourse.bass import ds, ts
from concourse.bass_utils import run_bass_kernel_spmd

B, L, D = 8, 2048, 512
P = 128
EPS = 1e-7
N_CORES = 8

f32 = mybir.dt.float32
f32r = mybir.dt.float32r
bf16 = mybir.dt.bfloat16
AF = mybir.ActivationFunctionType
ALU = mybir.AluOpType


def build_program(Lb=L, Db=D, n_cores=N_CORES):
    """Device program. Inputs: qT, kT feature-major [D, L]; v natural [L, D]."""
    NT = Lb // P          # 128-row tiles along q / k timesteps
    DC = Db // P          # 128-wide chunks of the feature dim
    KBW = 512             # k-block width (one PSUM bank of fp32)
    KB = Lb // KBW        # k blocks
    KT = KBW // P         # 128-wide k tiles per block
    QC = Lb // KBW        # 512-wide column chunks of qT
    C = 1.0 / math.sqrt(Db)

    nc = bacc.Bacc(
        "TRN2", target_bir_lowering=False, debug=False, num_devices=n_cores
    )
    qT = nc.dram_tensor("qT", [Db, Lb], bf16, kind="ExternalInput").ap()
    kT = nc.dram_tensor("kT", [Db, Lb], bf16, kind="ExternalInput").ap()
    v = nc.dram_tensor("v", [Lb, Db], f32r, kind="ExternalInput").ap()
    out = nc.dram_tensor("out", [Lb, Db], f32, kind="ExternalOutput").ap()

    with tile.TileContext(nc) as tc:
        with (
            tc.tile_pool(name="const", bufs=1) as cpool,
            tc.tile_pool(name="qTp", bufs=1) as qT_pool,
            tc.tile_pool(name="kTp", bufs=1) as kT_pool,
            tc.tile_pool(name="vSp", bufs=NT) as vS_pool,
            tc.tile_pool(name="warm", bufs=1) as warm_pool,
            tc.tile_pool(name="ep", bufs=6) as e_pool,
            tc.tile_pool(name="accp", bufs=3) as acc_pool,
            tc.tile_pool(name="outp", bufs=4) as out_pool,
            tc.tile_pool(name="scp", bufs=6) as sc_pool,
            tc.tile_pool(name="ps_s", bufs=3, space="PSUM") as ps_s,
            tc.tile_pool(name="ps_num", bufs=1, space="PSUM") as ps_num,
            tc.tile_pool(name="ps_tp", bufs=1, space="PSUM") as ps_tp,
        ):
            ones = cpool.tile([P, 1], f32, name="ones")
            nc.vector.memset(ones, 1.0)
            vmask = cpool.tile([P, NT], f32, name="vmask")

            # PE warm-up: ~4us of dummy fp32 matmuls flips the HAM clock gate
            # to full rate before real work arrives (fp32: 4 cycles/row, so a
            # handful of instructions covers the activity window)
            zf = warm_pool.tile([P, KBW], f32, name="zf")
            nc.vector.memset(zf, 0.0)
            wps = ps_tp.tile([P, KBW], f32, tag="tp", name="wps")
            for w in range(6):
                # all into one psum tile: pure WAW chain, no pool churn
                nc.tensor.matmul(wps, zf[:, :P], zf, start=True, stop=True)

            # Persistent SBUF residents, loaded straight from DRAM.
            # q/k column-chunk tiles [128, 512]: 2KB rows, good DMA shape.
            # kT loads issue on Sync's HWDGE ring, qT on ACT's ring, v on the
            # gpsimd SWDGE ring (casting f32 -> f32r) — three rings in parallel.
            # Each DMA ring sustains only ~120 GB/s, so tiles are assigned to
            # the three rings (Sync-HWDGE, ACT-HWDGE, gpsimd-SWDGE) in the
            # order the flash loop consumes them: k block 0 first, all of q
            # split across two rings (it gates every q-tile of k-block 0),
            # later k blocks last.
            qTs = {}
            kTs = {}

            def load_k(dc, c, eng):
                t_ = kT_pool.tile([P, KBW], bf16, tag=f"kT{dc}_{c}", name=f"kT{dc}_{c}")
                eng.dma_start(t_, kT[ds(dc * P, P), ds(c * KBW, KBW)])
                kTs[(dc, c)] = t_

            def load_q(dc, c, eng):
                t_ = qT_pool.tile([P, KBW], bf16, tag=f"qT{dc}_{c}", name=f"qT{dc}_{c}")
                eng.dma_start(t_, qT[ds(dc * P, P), ds(c * KBW, KBW)])
                qTs[(dc, c)] = t_

            vS_t = [None] * NT

            def load_v(t, eng):
                vt = vS_pool.tile([P, Db], f32r, tag="vS", name=f"vS{t}")
                eng.dma_start(vt, v[ts(t, P), :])
                vS_t[t] = vt
                nc.vector.tensor_reduce(
                    vmask[:, t : t + 1],
                    vt,
                    axis=mybir.AxisListType.X,
                    op=ALU.max,
                    apply_absolute_value=True,
                )

            # Both HWDGE engines share one physical ring (~230 GB/s) whose
            # first transfer lands only after the sync engine's ~8us
            # preamble. The gpsimd SWDGE ring (~100 GB/s) clears its
            # preamble at ~2us, so it bootstraps k block 0 and the first v
            # tiles; the HWDGE ring leads with q (which gates every q-tile
            # of k-block 0), then k block 1, the v tail, k blocks 2-3.
            v_head = min(8, NT)
            for dc in range(DC):
                load_k(dc, 0, nc.gpsimd)
            for c in range(QC):
                for dc in range(DC):
                    load_q(dc, c, nc.sync)
            if KB > 1:
                for dc in range(DC):
                    load_k(dc, 1, nc.sync)
            for t in range(v_head, NT):
                load_v(t, nc.sync)
            for c in range(2, KB):
                for dc in range(DC):
                    load_k(dc, c, nc.sync)
            for t in range(v_head):
                load_v(t, nc.gpsimd)
            # mask[k] = (max_d |v[k,d]|) > 0 -> {0.0, 1.0}; pm = mask * c
            nc.vector.tensor_scalar(vmask, vmask, 0.0, None, op0=ALU.is_gt)
            pm = cpool.tile([P, NT], f32, name="pm")
            nc.vector.tensor_scalar_mul(pm, vmask, C)

            def q_lhsT(qt, dc):
                return qTs[(dc, qt // KT)][:, ts(qt % KT, P)]

            # ---- Main flash loop over k blocks ----
            # The per-block epilogue (den, scale, writeback) is emitted inside
            # the NEXT block's first q-tile so its engine work interleaves
            # with the pipeline refill instead of stalling the PE on PSUM
            # slot reuse at every block boundary.
            def make_epilogue(kb, acc, nums):
                def emit():
                    for kt in range(KT):
                        j = kb * KT + kt
                        dps = ps_tp.tile([P, 1], f32, tag="tp", name=f"dps{j}")
                        nc.tensor.matmul(
                            dps, acc[:, ts(kt, P)], ones, start=True, stop=True
                        )
                        # scale = pm / (den * pm + EPS), pm = mask/sqrt(D)
                        scl = sc_pool.tile([P, 1], f32, tag="scl", name=f"scl{j}")
                        nc.vector.tensor_scalar(
                            scl, dps, pm[:, j : j + 1], EPS,
                            op0=ALU.mult, op1=ALU.add,
                        )
                        rcp = sc_pool.tile([P, 1], f32, tag="rcp", name=f"rcp{j}")
                        nc.vector.reciprocal(rcp, scl)
                        nc.vector.tensor_mul(rcp, rcp, pm[:, j : j + 1])
                        o = out_pool.tile([P, Db], f32, tag="o", name=f"o{j}")
                        nc.vector.tensor_scalar_mul(o, nums[kt], rcp)
                        nc.sync.dma_start(out[ts(j, P), :], o)
                return emit

            pending_epilogue = None
            for kb in range(KB):
                acc = acc_pool.tile([P, KBW], f32, tag="acc", name=f"acc{kb}")
                nums = None
                e_tiles = {}
                # software pipeline: stage-1 (scores+exp) runs one q-tile
                # ahead of stage-2 (E^T @ V) so the PE never waits on ACT
                for qt in range(NT + 1):
                    if qt < NT:
                        s_ps = ps_s.tile([P, KBW], f32, tag="s", name=f"s{kb}_{qt}")
                        for dc in range(DC):
                            nc.tensor.matmul(
                                s_ps,
                                q_lhsT(qt, dc),
                                kTs[(dc, kb)],
                                start=(dc == 0),
                                stop=(dc == DC - 1),
                            )
                        e = e_pool.tile([P, KBW], f32r, tag="e", name=f"e{kb}_{qt}")
                        nc.scalar.activation(e, s_ps, AF.Exp)
                        if qt == 0 and pending_epilogue is not None:
                            # previous block's den/scale/writeback lands here,
                            # after this block's first scores+exp are queued
                            pending_epilogue()
                            pending_epilogue = None
                        if qt == 0:
                            nc.vector.tensor_copy(acc, e)
                        else:
                            nc.vector.tensor_add(acc, acc, e)
                        e_tiles[qt] = e
                    if qt >= 1:
                        if nums is None:
                            # allocate after the previous block's release ops
                            # so the pool trace sees release before alloc
                            nums = [
                                ps_num.tile(
                                    [P, Db], f32,
                                    tag=f"num{kt}", name=f"num{kb}_{kt}",
                                )
                                for kt in range(KT)
                            ]
                        ep = e_tiles.pop(qt - 1)
                        for kt in range(KT):
                            nc.tensor.matmul(
                                nums[kt],
                                ep[:, ts(kt, P)],
                                vS_t[qt - 1],
                                start=(qt - 1 == 0),
                                stop=(qt - 1 == NT - 1),
                            )
                pending_epilogue = make_epilogue(kb, acc, nums)
            pending_epilogue()

    return nc


_cache = {}


def _get_compiled(Lb=L, Db=D):
    key = (Lb, Db)
    if key not in _cache:
        nc = build_program(Lb, Db)
        nc.compile()
        _cache[key] = nc
    return _cache[key]


def run(q, k, v, trace=False):
    nc = _get_compiled()
    q = np.ascontiguousarray(q, dtype=np.float32)
    k = np.ascontiguousarray(k, dtype=np.float32)
    v = np.ascontiguousarray(v, dtype=np.float32)
    import ml_dtypes

    in_maps = [
        {
            "qT": np.ascontiguousarray(q[i].T).astype(ml_dtypes.bfloat16),
            "kT": np.ascontiguousarray(k[i].T).astype(ml_dtypes.bfloat16),
            "v": v[i],
        }
        for i in range(N_CORES)
    ]
    res = run_bass_kernel_spmd(nc, in_maps, list(range(N_CORES)), trace=trace)
    out = np.stack([res.results[i]["out"] for i in range(N_CORES)], axis=0)
    return out.astype(np.float32, copy=False), res


def kernel(q, k, v):
    out, _ = run(q, k, v, trace=False)
    return out



# revision 3
# speedup vs baseline: 1.3960x; 1.3960x over previous
"""Bass/Tile TRN2 kernel for nn_LzScaleDotAttention (B=8, L=2048, D=512).

Math per batch b:
    S[q,k]   = sum_d Q[q,d] K[k,d]
    E        = exp(S)                       # inputs pre-scaled small, |S| < ~0.4
    num[k,d] = sum_q E[q,k] V[q,d]          # = E^T @ V
    den[k]   = sum_q E[q,k]
    mask[k]  = 1.0 if any(V[k,:] != 0) else 0.0
    out[k,d] = num[k,d] * mask[k]*c / (den[k]*mask[k]*c + EPS),  c = 1/sqrt(D)

fp8 formulation: both big matmuls run in fp8e4 DoubleRow mode (256-deep
contraction per instruction, 2x+ PE rate).  E ~= 1 +- 0.06 would lose all
information in e4m3 (0.125 steps near 1.0), so the kernel computes
t = tanh(S/2) = (E-1)/(E+1) ~= (E-1)/2 in one scalar-engine activation and
decomposes  num = Vsum + 2 * t^T V  (exact up to O(delta^2), which mostly
cancels in the renormalisation; measured ~5e-3 rel vs the 2e-2 budget).
Vsum = sum_q V[q,:] rides into each nums PSUM group as a rank-1 matmul
(0.5*ones[128,128] x Vpart) so no cross-partition broadcast is needed.
den = 2048 + 2*sum_q t, accumulated as bf16 DVE adds of the t tiles plus a
tiny ones-matmul per 128-wide k tile.

Sharding: batch dim (8) across the 8 NeuronCores, one batch per core (SPMD,
no collectives).  Host packs q/k feature-major fp8 [128, 4, 2048], v as fp8
q-tile pairs [128, 8, 1024] (for the DoubleRow rhs) and fp16 [2048, 512]
(for Vsum + mask).  Output returns bf16, widened to f32 on host.
"""

import math
import os
import sys

import numpy as np

for _p in ("/opt/trn_rl_repo", "/root/.axon_site/_ro/trn_rl_repo"):
    if os.path.isdir(_p) and _p not in sys.path:
        sys.path.append(_p)

import concourse.bacc as bacc
import concourse.mybir as mybir
import concourse.tile as tile
from concourse.bass import ds, ts
from concourse.bass_utils import run_bass_kernel_spmd

B, L, D = 8, 2048, 512
P = 128
EPS = 1e-7
N_CORES = 8

f32 = mybir.dt.float32
bf16 = mybir.dt.bfloat16
fp16 = mybir.dt.float16
fp8 = mybir.dt.float8e4
AF = mybir.ActivationFunctionType
ALU = mybir.AluOpType
DR = mybir.MatmulPerfMode.DoubleRow


def build_program(Lb=L, Db=D, n_cores=N_CORES):
    NT = Lb // P          # 16 q/k 128-row tiles
    NP = NT // 2          # 8 q-tile pairs (DoubleRow contraction granules)
    DC = Db // P          # 4 feature chunks of 128
    KBW = 512             # k-block width (one PSUM bank of fp32)
    KB = Lb // KBW        # 4 k blocks
    KT = KBW // P         # 4 k tiles per block
    QC = Lb // KBW        # 4 column chunks of q
    C = 1.0 / math.sqrt(Db)
    NQC = float(Lb) * C   # den constant term * c

    nc = bacc.Bacc(
        "TRN2", target_bir_lowering=False, debug=False, num_devices=n_cores
    )
    q8 = nc.dram_tensor("q8", [P, DC, Lb], fp8, kind="ExternalInput").ap()
    k8 = nc.dram_tensor("k8", [P, DC, Lb], fp8, kind="ExternalInput").ap()
    v8 = nc.dram_tensor("v8", [P, NP, 2 * Db], fp8, kind="ExternalInput").ap()
    v16 = nc.dram_tensor("v16", [Lb, Db], fp16, kind="ExternalInput").ap()
    out = nc.dram_tensor("out", [Lb, Db], bf16, kind="ExternalOutput").ap()

    with tile.TileContext(nc) as tc:
        with (
            tc.tile_pool(name="const", bufs=1) as cpool,
            tc.tile_pool(name="qp", bufs=1) as q_pool,
            tc.tile_pool(name="kp", bufs=1) as k_pool,
            tc.tile_pool(name="v8p", bufs=NP) as v8_pool,
            tc.tile_pool(name="v16p", bufs=NT) as v16_pool,
            tc.tile_pool(name="warm", bufs=1) as warm_pool,
            tc.tile_pool(name="t8p", bufs=3) as t8_pool,
            tc.tile_pool(name="accp", bufs=2) as acc_pool,
            tc.tile_pool(name="outp", bufs=4) as out_pool,
            tc.tile_pool(name="scp", bufs=6) as sc_pool,
            tc.tile_pool(name="ps_s", bufs=3, space="PSUM") as ps_s,
            tc.tile_pool(name="ps_num", bufs=1, space="PSUM") as ps_num,
            tc.tile_pool(name="ps_tp", bufs=1, space="PSUM") as ps_tp,
        ):
            ones_b = cpool.tile([P, 1], bf16, name="ones_b")
            nc.vector.memset(ones_b, 1.0)
            halfones = cpool.tile([P, P], fp16, name="halfones")
            nc.vector.memset(halfones, 0.5)
            vmask = cpool.tile([P, NT], f32, name="vmask")

            # PE warm-up: ~4us of dummy fp32 matmuls flips the HAM clock gate
            # to full rate before real work arrives
            zf = warm_pool.tile([P, KBW], f32, name="zf")
            nc.vector.memset(zf, 0.0)
            wps = ps_tp.tile([P, KBW], f32, tag="tp", name="wps")
            for w in range(6):
                nc.tensor.matmul(wps, zf[:, :P], zf, start=True, stop=True)

            # ---- DMA: gpsimd SWDGE ring bootstraps the critical path (it
            # clears its preamble ~2us; the sync HWDGE ring takes ~6-8us).
            qcs = [None] * QC
            kbs = [None] * KB
            v8t = [None] * NP
            v16t = [None] * NT

            def load_q(c, eng):
                t_ = q_pool.tile([P, DC, KBW], fp8, tag=f"q{c}", name=f"q{c}")
                eng.dma_start(t_, q8[:, :, ds(c * KBW, KBW)])
                qcs[c] = t_

            def load_k(kb, eng):
                t_ = k_pool.tile([P, DC, KBW], fp8, tag=f"k{kb}", name=f"k{kb}")
                eng.dma_start(t_, k8[:, :, ds(kb * KBW, KBW)])
                kbs[kb] = t_

            def load_v8(t, eng):
                t_ = v8_pool.tile([P, 2, Db], fp8, tag="v8", name=f"v8_{t}")
                eng.dma_start(t_, v8[:, ds(t, 1), :])
                v8t[t] = t_

            def load_v16(t, eng):
                t_ = v16_pool.tile([P, Db], fp16, tag="v16", name=f"v16_{t}")
                eng.dma_start(t_, v16[ts(t, P), :])
                v16t[t] = t_
                nc.vector.tensor_reduce(
                    vmask[:, t : t + 1],
                    t_,
                    axis=mybir.AxisListType.X,
                    op=ALU.max,
                    apply_absolute_value=True,
                )

            load_k(0, nc.gpsimd)
            load_q(0, nc.gpsimd)
            for t in range(NP):
                load_v8(t, nc.gpsimd)
            for c in range(1, QC):
                load_q(c, nc.sync)
            for t in range(NT):
                load_v16(t, nc.sync)
            load_k(1, nc.sync)
            load_k(2, nc.sync)
            load_k(3, nc.sync)

            # mask -> {0,1}; pm2 = 2*c*mask; npmeps = Lb*c*mask + EPS
            nc.vector.tensor_scalar(vmask, vmask, 0.0, None, op0=ALU.is_gt)
            pm2 = cpool.tile([P, NT], f32, name="pm2")
            nc.vector.tensor_scalar_mul(pm2, vmask, 2.0 * C)
            npmeps = cpool.tile([P, NT], f32, name="npmeps")
            nc.vector.tensor_scalar(
                npmeps, vmask, NQC, EPS, op0=ALU.mult, op1=ALU.add
            )

            # Vpart[p,d] = sum_t v16[t][p,d]  (fp16 partials; the rank-1
            # 0.5*ones matmul turns this into Vsum/2 broadcast in PSUM)
            vpart = cpool.tile([P, Db], fp16, name="vpart")
            nc.vector.tensor_copy(vpart, v16t[0])
            for t in range(1, NT):
                nc.vector.tensor_add(vpart, vpart, v16t[t])

            # ---- Main flash loop over k blocks ----
            def make_epilogue(kb, acc, nums):
                def emit():
                    for kt in range(KT):
                        j = kb * KT + kt
                        dps = ps_tp.tile([P, 1], f32, tag="tp", name=f"dps{j}")
                        nc.tensor.matmul(
                            dps, acc[:, ts(kt, P)], ones_b, start=True, stop=True
                        )
                        # scl = den*pm + EPS = dps*(2*c*mask) + (L*c*mask+EPS)
                        scl = sc_pool.tile([P, 1], f32, tag="scl", name=f"scl{j}")
                        nc.vector.tensor_scalar(
                            scl, dps, pm2[:, j : j + 1], npmeps[:, j : j + 1],
                            op0=ALU.mult, op1=ALU.add,
                        )
                        rcp = sc_pool.tile([P, 1], f32, tag="rcp", name=f"rcp{j}")
                        nc.vector.reciprocal(rcp, scl)
                        nc.vector.tensor_mul(rcp, rcp, pm2[:, j : j + 1])
                        o = out_pool.tile([P, Db], bf16, tag="o", name=f"o{j}")
                        nc.vector.tensor_scalar_mul(o, nums[kt], rcp)
                        nc.sync.dma_start(out[ts(j, P), :], o)
                return emit

            pending_epilogue = None
            for kb in range(KB):
                acc = acc_pool.tile([P, KBW], bf16, tag="acc", name=f"acc{kb}")
                nums = None
                t8_tiles = {}
                # stage-1 (scores+tanh) runs one pair ahead of stage-2
                # (t^T @ V DoubleRow) so the PE never waits on ACT
                for qt in range(NT + 2):
                    if qt < NT:
                        c, qq = qt // 4, qt % 4
                        s_ps = ps_s.tile([P, KBW], f32, tag="s", name=f"s{kb}_{qt}")
                        nc.tensor.matmul(
                            s_ps,
                            qcs[c][:, ds(0, 2), ts(qq, P)],
                            kbs[kb][:, ds(0, 2), :],
                            start=True, stop=False, perf_mode=DR,
                        )
                        nc.tensor.matmul(
                            s_ps,
                            qcs[c][:, ds(2, 2), ts(qq, P)],
                            kbs[kb][:, ds(2, 2), :],
                            start=False, stop=True, perf_mode=DR,
                        )
                        pr, par = qt // 2, qt % 2
                        if par == 0:
                            t8 = t8_pool.tile(
                                [P, 2, KBW], fp8, tag="t8", name=f"t8_{kb}_{pr}"
                            )
                            t8_tiles[pr] = t8
                        t8 = t8_tiles[pr]
                        nc.scalar.activation(
                            t8[:, ds(par, 1), :], s_ps, AF.Tanh, scale=0.5
                        )
                        if qt == 0 and pending_epilogue is not None:
                            pending_epilogue()
                            pending_epilogue = None
                        if qt == 0:
                            nc.vector.tensor_copy(acc, t8[:, ds(0, 1), :])
                        else:
                            nc.vector.tensor_add(acc, acc, t8[:, ds(par, 1), :])
                    # stage 2: pair pr2 = (qt-2)//2 is complete
                    if qt >= 2 and qt % 2 == 0:
                        pr2 = (qt - 2) // 2
                        if nums is None:
                            nums = [
                                ps_num.tile(
                                    [P, Db], f32,
                                    tag=f"num{kt}", name=f"num{kb}_{kt}",
                                )
                                for kt in range(KT)
                            ]
                        tp = t8_tiles.pop(pr2)
                        for kt in range(KT):
                            nc.tensor.matmul(
                                nums[kt],
                                tp[:, :, ts(kt, P)],
                                v8t[pr2],
                                start=(pr2 == 0), stop=False,
                                perf_mode=DR,
                            )
                # rank-1 Vsum/2 broadcast closes each nums accumulation group
                for kt in range(KT):
                    nc.tensor.matmul(
                        nums[kt], halfones, vpart, start=False, stop=True
                    )
                pending_epilogue = make_epilogue(kb, acc, nums)
            pending_epilogue()

    return nc


_cache = {}


def _get_compiled(Lb=L, Db=D):
    key = (Lb, Db)
    if key not in _cache:
        nc = build_program(Lb, Db)
        nc.compile()
        _cache[key] = nc
    return _cache[key]


def run(q, k, v, trace=False):
    nc = _get_compiled()
    q = np.ascontiguousarray(q, dtype=np.float32)
    k = np.ascontiguousarray(k, dtype=np.float32)
    v = np.ascontiguousarray(v, dtype=np.float32)
    import ml_dtypes

    f8 = ml_dtypes.float8_e4m3

    def pack_qk(x):
        # [L, D] -> [128, DC, L] fp8, element (p, ch, j) = x[j, ch*128+p]
        return np.ascontiguousarray(
            x.T.reshape(D // P, P, L).transpose(1, 0, 2)
        ).astype(f8)

    def pack_v8(x):
        # [L, D] -> [128, NP, 2D]: (p, t, par*512+d) = x[t*256+par*128+p, d]
        return np.ascontiguousarray(
            x.reshape(L // 256, 2, P, D).transpose(2, 0, 1, 3).reshape(P, L // 256, 2 * D)
        ).astype(f8)

    in_maps = [
        {
            "q8": pack_qk(q[i]),
            "k8": pack_qk(k[i]),
            "v8": pack_v8(v[i]),
            "v16": v[i].astype(np.float16),
        }
        for i in range(N_CORES)
    ]
    res = run_bass_kernel_spmd(nc, in_maps, list(range(N_CORES)), trace=trace)
    out = np.stack([res.results[i]["out"] for i in range(N_CORES)], axis=0)
    return out.astype(np.float32), res


def kernel(q, k, v):
    out, _ = run(q, k, v, trace=False)
    return out
